# revision 13
# baseline (speedup 1.0000x reference)
"""Conv4d via 1D-Winograd F(2,3) along w, on 8 TRN2 NeuronCores.

Per output pair (t,t+1) the direct bf16 scheme needs 432 N=512 matmuls; the
w-axis Winograd transform replaces the 3 kw-taps by 4 pointwise products on
half the w-resolution: 288 matmuls -> 246us PE floor instead of 368us.

  input transform  (DVE/Pool, bf16):  per cube, coeffs c0..c3 over (d,h,t8):
      c0 = x[2t]-x[2t+2], c1 = x[2t+1]+x[2t+2],
      c2 = x[2t+2]-x[2t+1], c3 = x[2t+1]-x[2t+3]
  coeff GEMMs (PE): per pair, per point c, the same (j,ci)x(u,co) banded
      L/H time-block structure as the direct kernel, over taps (kd,kh);
      out m_c[(u,co), (d4,h16,t8)] accumulates 18 matmuls in PSUM
  points processed in halves {0,1} then {2,3}; m0/m1 evacuated to SBUF as
      a01 = copy(m0) (Act) and a1b = m1+bias (DVE), then with m2/m3 still
      in PSUM the inverse (one PSUM operand per op — walrus rejects two):
      y[2t]   = (a01 + m2) + a1b           (DVE, then Pool)
      y[2t+1] = (a1b - m3) - m2            (DVE x2)
  all strided window reads/writes use unit-stride parity-split views
  (w = 2*tt+par); stepped (::2) APs also crash walrus codegen
"""
import numpy as np

B, C, S, KW = 4, 64, 16, 3
SP = S + 2
RCUBE = SP * SP * SP       # raw padded cube 18^3
TQ = S // 2                # 8 wino tiles per row
CCUBE = SP * SP * TQ       # one coeff point-cube: (d18, h18, t8)
NCORES = 8
TSH = S * B // NCORES

_PROGRAM = None


def _build_program():
    import concourse.bacc as bacc
    import concourse.mybir as mybir
    import concourse.tile as tile

    nc = bacc.Bacc("TRN2", target_bir_lowering=False, debug=False,
                   num_devices=NCORES)
    bf16 = mybir.dt.bfloat16
    f32 = mybir.dt.float32
    IDENT = mybir.ActivationFunctionType.Identity

    xs_d = nc.dram_tensor("xs", [5, 4, 128, CCUBE], bf16,
                          kind="ExternalInput").ap()
    wl_d = nc.dram_tensor("wl", [128, 36 * 128], bf16, kind="ExternalInput").ap()
    wh_d = nc.dram_tensor("wh", [128, 36 * 128], bf16, kind="ExternalInput").ap()
    bias_d = nc.dram_tensor("bias2", [128, 1], f32, kind="ExternalInput").ap()
    y_d = nc.dram_tensor("y", [TSH, C, S * S * S], bf16,
                         kind="ExternalOutput").ap()

    with tile.TileContext(nc) as tc:
        with (
            tc.tile_pool(name="xc", bufs=5) as cpool,
            tc.tile_pool(name="wp", bufs=1) as wpool,
            tc.tile_pool(name="ev", bufs=2) as epool,
            tc.tile_pool(name="st", bufs=2) as spool,
            tc.tile_pool(name="ps", bufs=8, space="PSUM") as pspool,
        ):
            wlt = wpool.tile([128, 36 * 128], bf16)
            wht = wpool.tile([128, 36 * 128], bf16)
            bias_t = wpool.tile([128, 1], f32)
            # coeff cubes: [128, (c4, d18, h18, t8)] — rotating 3-slot
            # pool: pair u reads cubes u and u+1 only
            vts = []

            wpiece = 9 * 128

            def wdma(q, t, p):
                lo, hi = p * wpiece, (p + 1) * wpiece
                q.dma_start(t[:, lo:hi], (wl_d if t is wlt else wh_d)[:, lo:hi])

            wdma(nc.gpsimd, wlt, 0)

            hcc = CCUBE // 2

            def load(k, queues):
                vts.append([cpool.tile([128, CCUBE], bf16, name=f"vt{c}")
                            for c in range(4)])
                for c in range(4):
                    q0 = queues[c % len(queues)]
                    q1 = queues[(c + 1) % len(queues)]
                    q0.dma_start(vts[k][c][:, 0:hcc], xs_d[k][c][:, 0:hcc])
                    q1.dma_start(vts[k][c][:, hcc:], xs_d[k][c][:, hcc:])

            vts.append([cpool.tile([128, CCUBE], bf16, name=f"vt{c}")
                        for c in range(4)])
            nc.sync.dma_start(vts[0][0][:, 0:hcc], xs_d[0][0][:, 0:hcc])
            nc.gpsimd.dma_start(vts[0][0][:, hcc:], xs_d[0][0][:, hcc:])
            nc.sync.dma_start(vts[0][1][:, 0:hcc], xs_d[0][1][:, 0:hcc])
            nc.sync.dma_start(vts[0][1][:, hcc:], xs_d[0][1][:, hcc:])
            nc.gpsimd.dma_start(bias_t[:], bias_d)
            nc.sync.dma_start(vts[0][2][:], xs_d[0][2])
            nc.gpsimd.dma_start(vts[0][3][:], xs_d[0][3])
            for p in (1, 2, 3):
                wdma(nc.gpsimd, wlt, p)
            load(1, (nc.sync,))
            for p in range(4):
                wdma(nc.gpsimd, wht, p)
            load(2, (nc.sync, nc.scalar))
            load(3, (nc.scalar, nc.sync))
            load(4, (nc.sync, nc.scalar))

            def vvs(k, c):
                return vts[k][c].rearrange("p (d ht) -> p d ht",
                                           d=SP, ht=SP * TQ)

            for u in range(TSH // 2):  # output pair
                ystage = spool.tile([128, S * S * S], bf16, name="ystage")
                # parity-split layout (d, h, par, t): host re-interleaves w
                yv2 = ystage.rearrange("p (d h par t) -> p d h par t",
                                       d=S, h=S, par=2, t=TQ)
                a01 = epool.tile([128, 4 * 512], bf16, name="a01")
                a1b = epool.tile([128, 4 * 512], bf16, name="a1b")
                for half in range(2):
                    banks = [pspool.tile([128, 512], f32, name="bank")
                             for _ in range(8)]
                    for blk in range(2):   # L then H
                        wt = wlt if blk == 0 else wht
                        for ph in range(2):
                            c = half * 2 + ph
                            for kd in range(KW):
                                for kh in range(KW):
                                    iw = c * 9 + kd * KW + kh
                                    lhsT = wt[:, iw * 128:(iw + 1) * 128]
                                    for dq in range(4):
                                        rhs = vvs(u + blk, c)[
                                            :,
                                            4 * dq + kd:4 * dq + kd + 4,
                                            kh * TQ:(kh + S) * TQ]
                                        nc.tensor.matmul(
                                            banks[ph * 4 + dq][:], lhsT, rhs,
                                            start=(blk == 0 and kd == 0
                                                   and kh == 0),
                                            stop=(blk == 1 and kd == 2
                                                  and kh == 2),
                                        )
                    if half == 0:
                        for dq in range(4):
                            sl = slice(dq * 512, (dq + 1) * 512)
                            nc.scalar.activation(a01[:, sl], banks[dq][:],
                                                 IDENT)
                            nc.vector.tensor_scalar_add(a1b[:, sl],
                                                        banks[4 + dq][:],
                                                        bias_t[:])
                    else:
                        yqs = (nc.sync, nc.scalar, nc.gpsimd)
                        t02s, tas = [], []
                        # m2 (ph0) banks stop 36 matmuls before m3 (ph1):
                        # everything that only needs m2 runs under the m3
                        # matmuls, leaving just y1 = tA - m3 for the tail
                        for dq in range(4):
                            sl = slice(dq * 512, (dq + 1) * 512)
                            t02 = epool.tile([128, 512], bf16,
                                             name=f"t02_{dq}")
                            nc.vector.tensor_add(t02[:], a01[:, sl],
                                                 banks[dq][:])
                            t02s.append(t02)
                            ta = epool.tile([128, 512], bf16,
                                            name=f"ta_{dq}")
                            nc.vector.tensor_sub(ta[:], a1b[:, sl],
                                                 banks[dq][:])
                            tas.append(ta)
                        for dq in range(4):
                            sl = slice(dq * 512, (dq + 1) * 512)
                            y0 = yv2[:, 4 * dq:4 * dq + 4, :, 0:1, :]
                            y1 = yv2[:, 4 * dq:4 * dq + 4, :, 1:2, :]
                            nc.gpsimd.tensor_add(y0, t02s[dq][:], a1b[:, sl])
                            nc.vector.tensor_sub(y1, tas[dq][:],
                                                 banks[4 + dq][:])
                            cs = slice(dq * 1024, (dq + 1) * 1024)
                            yqs[(2 * dq) % 3].dma_start(
                                y_d[2 * u][:, cs], ystage[0:C, cs])
                            yqs[(2 * dq + 1) % 3].dma_start(
                                y_d[2 * u + 1][:, cs], ystage[C:128, cs])

    nc.compile()
    return nc


def _host_prep(x, weight, bias):
    import ml_dtypes

    xpad = np.pad(x, ((0, 0), (0, 0), (0, 0), (1, 1), (1, 1), (1, 1)),
                  mode="wrap").astype(np.float32)  # (B,C,S,18,18,18)
    # host-side Winograd input transform along w (4 adds/elem; the 5184
    # MACs/elem contraction stays on device):
    xa = xpad[..., 0:16:2]
    xb = xpad[..., 1:17:2]
    xc = xpad[..., 2:18:2]
    xd3 = xpad[..., 3::2]
    # (4, B, C, S, 18, 18, 8)
    xw = np.stack([xa - xc, xb + xc, xc - xb, xb - xd3]).astype(
        ml_dtypes.bfloat16)

    # wino-transformed weights: point c from kw-taps (correlation form)
    #   g0 = w0 ; g1 = (w0+w1+w2)/2 ; g2 = (w0-w1+w2)/2 ; g3 = w2
    w = weight.astype(np.float32)  # (3, co, ci, kd, kh, kw)
    gw = np.stack([
        w[..., 0],
        0.5 * (w[..., 0] + w[..., 1] + w[..., 2]),
        0.5 * (w[..., 0] - w[..., 1] + w[..., 2]),
        w[..., 2],
    ], axis=-1)  # (3, co, ci, kd, kh, c4)

    wl = np.zeros((128, 36, 128), dtype=np.float32)
    wh = np.zeros((128, 36, 128), dtype=np.float32)
    for c in range(4):
        for kd in range(KW):
            for kh in range(KW):
                iw = c * 9 + kd * KW + kh
                for j in range(2):
                    for u in range(2):
                        gl = j - u
                        if 0 <= gl < KW:
                            wl[j * C:(j + 1) * C, iw, u * C:(u + 1) * C] = \
                                gw[gl, :, :, kd, kh, c].T
                        gh = j - u + 2
                        if 0 <= gh < KW:
                            wh[j * C:(j + 1) * C, iw, u * C:(u + 1) * C] = \
                                gw[gh, :, :, kd, kh, c].T
    wl = wl.reshape(128, 36 * 128).astype(ml_dtypes.bfloat16)
    wh = wh.reshape(128, 36 * 128).astype(ml_dtypes.bfloat16)
    bias2 = np.concatenate([bias, bias]).astype(np.float32).reshape(128, 1)

    in_maps = []
    for core in range(NCORES):
        b = core // 2
        t0 = TSH * (core % 2)
        xs = np.empty((5, 4, 128, CCUBE), dtype=ml_dtypes.bfloat16)
        for k in range(5):
            sa = (t0 - 1 + 2 * k) % S
            sb = (t0 + 2 * k) % S
            for c in range(4):
                xs[k, c, 0:C] = xw[c, b, :, sa].reshape(C, CCUBE)
                xs[k, c, C:128] = xw[c, b, :, sb].reshape(C, CCUBE)
        in_maps.append({"xs": xs, "wl": wl, "wh": wh, "bias2": bias2})
    return in_maps


LAST_RESULTS = None


def kernel(x, weight, bias, _trace=False):
    global _PROGRAM, LAST_RESULTS
    from concourse import bass_utils

    x = np.asarray(x, dtype=np.float32)
    weight = np.asarray(weight, dtype=np.float32)
    bias = np.asarray(bias, dtype=np.float32)

    if _PROGRAM is None:
        _PROGRAM = _build_program()
    nc = _PROGRAM

    in_maps = _host_prep(x, weight, bias)
    res = bass_utils.run_bass_kernel_spmd(
        nc, in_maps, core_ids=list(range(NCORES)), trace=_trace
    )
    LAST_RESULTS = res

    out = np.empty((B, C, S, S, S, S), dtype=np.float32)
    for core in range(NCORES):
        b = core // 2
        t0 = TSH * (core % 2)
        y = np.asarray(res.results[core]["y"], dtype=np.float32)
        # device layout per slice: (d, h, par, t) -> w = 2t + par
        y = y.reshape(TSH, C, S, S, 2, TQ).transpose(0, 1, 2, 3, 5, 4)
        out[b, :, t0:t0 + TSH] = y.reshape(TSH, C, S, S, S).transpose(
            1, 0, 2, 3, 4)
    return out
